# revision 14
# baseline (speedup 1.0000x reference)
"""3-layer GCN (100k nodes, 1.6M edges, 128->128->128->40) on 8 trn2 cores.

Self-contained harness kernel: kernel(**inputs) takes the FULL unsharded
inputs and returns the FULL [100000, 40] float32 output.

Strategy (1D node partition, edges sharded by dst):
  - nodes split contiguously across the 8 cores (12500 each, padded 12544 =
    98 windows of 128); edges assigned to the core owning their dst.
  - the gathered table holds RAW scaled node rows tab_l[i] = bf16(h_l[i] *
    ns[i] [* nd[i] for l>0]), 128 bf16 = 256B per row; the weight matmul is
    applied AFTER aggregation (W commutes with segment_sum), so every layer
    gathers 256B rows and the per-node transform pass disappears.
  - tab_0 is computed host-side and fed as a replicated ExternalInput, so
    there is no AllGather before layer 0; layers 1/2 need one monolithic
    bf16 AllGather each (3.2MB in / 25.7MB out).
  - aggregation: edges grouped by 128-wide dst windows (GW windows per
    gather group, 4 int16 buckets of 25088 table rows); per-(window,bucket)
    runs are padded to 16 slots (the SWDGE descriptor granularity) so the
    queue-rate-bound gather wastes little; 128-slot gather columns may span
    adjacent windows -- each window matmuls every column its run overlaps,
    with its own one-hot S instance (built on DVE from compile-time dst
    metadata; foreign slots get the -1 sentinel and route nowhere).
  - window epilogue: PSUM->SBUF copy, f32 matmul by W, then one ACT
    relu-with-scale (nd*ns folded; biases are zero per the problem spec)
    emits the next layer's bf16 table rows node-major.
"""
import sys
sys.path.insert(0, '/opt/trn_rl_repo')

import numpy as np

import concourse.bass as bass
import concourse.bacc as bacc
import concourse.tile as tile
import concourse.mybir as mybir
from concourse.bass_utils import run_bass_kernel_spmd

f32 = mybir.dt.float32
bf16 = mybir.dt.bfloat16
i16 = mybir.dt.int16

NC = 8
GW = 4                 # dst windows per gather group
N_NODES = 100000
SHARD = N_NODES // NC          # 12500
NW = (SHARD + 127) // 128      # 98
PADSHARD = NW * 128            # 12544
NPAD = NC * PADSHARD           # 100352
NBUK = 4
BUKSZ = 25088                  # NPAD / 4, int16-addressable


def _preprocess(src, dst):
    src = np.asarray(src).astype(np.int64)
    dst = np.asarray(dst).astype(np.int64)

    outdeg = np.bincount(src, minlength=N_NODES)
    indeg = np.bincount(dst, minlength=N_NODES)
    ns = (1.0 / np.sqrt(np.maximum(outdeg, 1))).astype(np.float32)
    nd = (1.0 / np.sqrt(np.maximum(indeg, 1))).astype(np.float32)

    srcg = (src // SHARD) * PADSHARD + (src % SHARD)
    b_s = srcg // BUKSZ
    reb_s = srcg - b_s * BUKSZ
    ecore = dst // SHARD

    cores = []
    counts = np.zeros((NC, NW, NBUK), dtype=np.int64)
    for c in range(NC):
        m = ecore == c
        ld = dst[m] - c * SHARD
        w = ld >> 7
        slot_d = ld & 127
        b = b_s[m]
        reb = reb_s[m]
        order = np.lexsort((reb, b, w))
        w, b, slot_d, reb = w[order], b[order], slot_d[order], reb[order]
        key = w * NBUK + b
        cnt = np.bincount(key, minlength=NW * NBUK).reshape(NW, NBUK)
        counts[c] = cnt
        cores.append((w, b, slot_d, reb, key))

    # per-(w,b) slot counts (exact union-max across cores; runs start at
    # arbitrary slot offsets, shared columns are handled via S instances)
    C16 = counts.max(axis=0)

    NG = (NW + GW - 1) // GW
    group_ws = [list(range(g * GW, min((g + 1) * GW, NW))) for g in range(NG)]

    # per-(g,b): idx offset, padded call length (mult of 128), n_cols;
    # per-(w,b): slot start within its (g,b) call
    start16 = np.zeros((NW, NBUK), dtype=np.int64)
    calls = []          # [g][b] = (idx_off, n_call, n_cols) or None
    idx_off = 0
    for g, ws in enumerate(group_ws):
        gcalls = []
        for b in range(NBUK):
            acc = 0
            for w in ws:
                start16[w, b] = acc
                acc += int(C16[w, b])
            if acc == 0:
                gcalls.append(None)
                continue
            n_call = ((acc + 127) // 128) * 128
            gcalls.append((idx_off, n_call, n_call // 128))
            idx_off += n_call
        calls.append(gcalls)
    TOTSLOT = idx_off

    # per-window matmul chunk list: (bucket, physical col in G_b) pairs,
    # and the window's S instance count
    mm_list = []        # [w] = [(b, pc), ...]
    for g, ws in enumerate(group_ws):
        for w in ws:
            lst = []
            for b in range(NBUK):
                if int(C16[w, b]) == 0 or calls[g][b] is None:
                    continue
                s0 = int(start16[w, b])
                s1 = s0 + int(C16[w, b])
                for pc in range(s0 // 128, (s1 - 1) // 128 + 1):
                    lst.append((b, pc))
            mm_list.append(lst)
    C_w = np.array([len(lst) for lst in mm_list], dtype=np.int64)
    colbase_w = np.zeros(NW, dtype=np.int64)
    colbase_w[1:] = np.cumsum(C_w)[:-1]
    TOTINST = int(C_w.sum())

    # dstl instance-column lookup: for (w, b, pc) -> instance col
    inst_of = {}
    for w in range(NW):
        for k, (b, pc) in enumerate(mm_list[w]):
            inst_of[(w, b, pc)] = int(colbase_w[w]) + k

    per_core = []
    for c in range(NC):
        w, b, slot_d, reb, key = cores[c]
        run_start = np.zeros(NW * NBUK, dtype=np.int64)
        run_start[1:] = np.cumsum(np.bincount(key, minlength=NW * NBUK))[:-1]
        p = np.arange(len(key)) - run_start[key]

        g_of_w = w // GW
        call_off = np.array([[calls[g_of][b_] [0] if calls[g_of][b_] else 0
                              for b_ in range(NBUK)]
                             for g_of in range(NG)], dtype=np.int64)
        s = start16[w, b] + p                  # slot within the (g,b) call
        ipos = call_off[g_of_w, b] + s
        pc = s // 128
        sslot = s % 128

        inst = np.array([inst_of[(int(w_), int(b_), int(pc_))]
                         for w_, b_, pc_ in zip(w, b, pc)], dtype=np.int64)

        idx_flat = np.zeros(TOTSLOT, dtype=np.int16)
        idx_flat[ipos] = reb.astype(np.int16)
        dstl = np.full((128, TOTINST), -1.0, dtype=np.float32)
        dstl[sslot, inst] = slot_d.astype(np.float32)

        idx2d = np.tile(idx_flat.reshape(TOTSLOT // 16, 16).T, (8, 1)).copy()

        nsnd_sh = np.zeros(PADSHARD, dtype=np.float32)
        nd_sh = np.zeros(PADSHARD, dtype=np.float32)
        nsnd_sh[:SHARD] = (ns * nd)[c * SHARD:(c + 1) * SHARD]
        nd_sh[:SHARD] = nd[c * SHARD:(c + 1) * SHARD]
        nsndcol = nsnd_sh.reshape(NW, 128).T.copy()
        ndcol = nd_sh.reshape(NW, 128).T.copy()

        per_core.append(dict(dstl=dstl, idx=idx2d,
                             nsndcol=nsndcol, ndcol=ndcol))

    st = dict(C_w=C_w, TOTINST=TOTINST, TOTSLOT=TOTSLOT,
              colbase_w=colbase_w, group_ws=group_ws, calls=calls,
              mm_list=mm_list, ns=ns, nd=nd)
    return st, per_core


def _build_program(st, f_cls):
    C_w = st['C_w']
    TOTINST, TOTSLOT = st['TOTINST'], st['TOTSLOT']
    colbase_w = st['colbase_w']
    group_ws, calls, mm_list = st['group_ws'], st['calls'], st['mm_list']
    fcp = 64 * ((f_cls + 63) // 64)

    nc = bacc.Bacc(None, target_bir_lowering=False, num_swdge_queues=NBUK)

    hp0_d = nc.dram_tensor("hp0full", [NPAD, 128], bf16, kind="ExternalInput")
    idx_d = nc.dram_tensor("idx16", [128, TOTSLOT // 16], i16, kind="ExternalInput")
    dstl_d = nc.dram_tensor("dstl", [128, TOTINST], bf16, kind="ExternalInput")
    iota_d = nc.dram_tensor("iota", [128, 128], bf16, kind="ExternalInput")
    nsnd_d = nc.dram_tensor("nsndcol", [128, NW], f32, kind="ExternalInput")
    ndcol_d = nc.dram_tensor("ndcol", [128, NW], f32, kind="ExternalInput")
    W0_d = nc.dram_tensor("W0", [128, 128], f32, kind="ExternalInput")
    W1_d = nc.dram_tensor("W1", [128, 128], f32, kind="ExternalInput")
    W2_d = nc.dram_tensor("W2p", [128, fcp], f32, kind="ExternalInput")
    b2_d = nc.dram_tensor("b2rep", [128, fcp], f32, kind="ExternalInput")
    out_d = nc.dram_tensor("out", [SHARD, f_cls], f32, kind="ExternalOutput")

    hp1_own = nc.dram_tensor("hp1_own", [PADSHARD, 128], bf16)
    hp2_own = nc.dram_tensor("hp2_own", [PADSHARD, 128], bf16)
    hp1_full = nc.dram_tensor("hp1_full", [NPAD, 128], bf16, addr_space="Shared")
    hp2_full = nc.dram_tensor("hp2_full", [NPAD, 128], bf16, addr_space="Shared")

    rg = [list(range(NC))]

    with tile.TileContext(nc) as tc:
        with (
            tc.tile_pool(name="const", bufs=1) as cpool,
            tc.tile_pool(name="gpool", bufs=7) as gpool,
            tc.tile_pool(name="spool", bufs=6) as spool,
            tc.tile_pool(name="xpool", bufs=6) as xpool,
            tc.tile_pool(name="ipool", bufs=4) as ipool,
            tc.tile_pool(name="psA", bufs=3, space="PSUM") as psA,
            tc.tile_pool(name="psC", bufs=5, space="PSUM") as psC,
        ):
            sW0 = cpool.tile([128, 128], f32); nc.sync.dma_start(sW0[:], W0_d[:])
            sW1 = cpool.tile([128, 128], f32); nc.sync.dma_start(sW1[:], W1_d[:])
            sW2 = cpool.tile([128, fcp], f32); nc.sync.dma_start(sW2[:], W2_d[:])
            sb2 = cpool.tile([128, fcp], f32); nc.sync.dma_start(sb2[:], b2_d[:])
            siota = cpool.tile([128, 128], bf16); nc.sync.dma_start(siota[:], iota_d[:])
            sdstl = cpool.tile([128, TOTINST], bf16); nc.sync.dma_start(sdstl[:], dstl_d[:])
            snsnd = cpool.tile([128, NW], f32); nc.sync.dma_start(snsnd[:], nsnd_d[:])
            sndcol = cpool.tile([128, NW], f32); nc.sync.dma_start(sndcol[:], ndcol_d[:])

            def agg_layer(hp_full, layer):
                sW = (sW0, sW1, sW2)[layer]
                fo = 128 if layer < 2 else fcp
                hp_next_own = (hp1_own, hp2_own, None)[layer]
                for g, ws in enumerate(group_ws):
                    gcalls = calls[g]
                    idxcols = sum(c[1] // 16 for c in gcalls if c is not None)
                    idxs = ipool.tile([128, max(1, idxcols)], i16, tag="idx")
                    g0 = next(c for c in gcalls if c is not None)[0]
                    nc.sync.dma_start(
                        idxs[:, 0:idxcols],
                        idx_d[:, g0 // 16:(g0 + idxcols * 16) // 16])
                    Gt = []
                    loff = 0
                    for b in range(NBUK):
                        if gcalls[b] is None:
                            Gt.append(None)
                            continue
                        off, n_call, n_cols = gcalls[b]
                        G = gpool.tile([128, n_cols, 128], bf16, tag=f"G{b}")
                        nc.gpsimd.dma_gather(
                            out_ap=G[:, :, :],
                            in_ap=hp_full[b * BUKSZ:(b + 1) * BUKSZ, :],
                            idxs_ap=idxs[:16, loff:loff + n_call // 16],
                            num_idxs=n_call,
                            num_idxs_reg=n_call,
                            elem_size=128,
                            single_packet=False,
                            queue_num=b,
                        )
                        Gt.append(G)
                        loff += n_call // 16
                    for w in ws:
                        cw = int(C_w[w])
                        cb = int(colbase_w[w])
                        S = spool.tile([128, cw * 128], bf16, tag="S")
                        in0 = sdstl[:, cb:cb + cw].unsqueeze(2).broadcast_to([128, cw, 128])
                        in1 = siota[:, :].unsqueeze(1).broadcast_to([128, cw, 128])
                        nc.vector.tensor_tensor(
                            S[:, :].rearrange("p (c x) -> p c x", x=128),
                            in0, in1, mybir.AluOpType.is_equal)
                        ps = psC.tile([128, 128], f32, tag="psC")
                        for k, (b, pc) in enumerate(mm_list[w]):
                            nc.tensor.matmul(
                                ps[:], Gt[b][:, pc, :],
                                S[:, k * 128:(k + 1) * 128],
                                start=(k == 0), stop=(k == cw - 1))
                        aggT = xpool.tile([128, 128], f32, tag="aggT")
                        nc.scalar.activation(aggT[:], ps[:],
                                             mybir.ActivationFunctionType.Copy)
                        ps2 = psA.tile([128, fo], f32, tag="psA")
                        nc.tensor.matmul(ps2[:], aggT[:], sW[:], start=True, stop=True)
                        if layer < 2:
                            hp = xpool.tile([128, 128], bf16, tag="hp")
                            nc.scalar.activation(hp[:], ps2[:],
                                                 mybir.ActivationFunctionType.Relu,
                                                 scale=snsnd[:, w:w + 1])
                            nc.sync.dma_start(hp_next_own[w * 128:(w + 1) * 128, :], hp[:])
                        else:
                            t = xpool.tile([128, fcp], f32, tag="t2")
                            nc.vector.tensor_scalar_mul(t[:], ps2[:], sndcol[:, w:w + 1])
                            o = xpool.tile([128, fcp], f32, tag="o2")
                            nc.vector.tensor_tensor(o[:], t[:], sb2[:],
                                                    mybir.AluOpType.add)
                            rows = min(128, SHARD - w * 128)
                            nc.sync.dma_start(out_d[w * 128:w * 128 + rows, :],
                                              o[:rows, 0:f_cls])

            agg_layer(hp0_d, 0)
            nc.gpsimd.collective_compute("AllGather", mybir.AluOpType.bypass, rg,
                                         ins=[hp1_own[:, :]], outs=[hp1_full[:, :]])
            agg_layer(hp1_full, 1)
            nc.gpsimd.collective_compute("AllGather", mybir.AluOpType.bypass, rg,
                                         ins=[hp2_own[:, :]], outs=[hp2_full[:, :]])
            agg_layer(hp2_full, 2)

    nc.compile()
    return nc


_cache = {}


def kernel(feat, src, dst, W0, b0, W1, b1, W2, b2):
    import ml_dtypes
    feat = np.ascontiguousarray(feat, dtype=np.float32)
    f_cls = np.asarray(W2).shape[1]
    fcp = 64 * ((f_cls + 63) // 64)

    key = (hash(np.asarray(src).tobytes()), hash(np.asarray(dst).tobytes()))
    if key in _cache:
        st, per_core, nc_prog = _cache[key]
    else:
        st, per_core = _preprocess(src, dst)
        nc_prog = _build_program(st, f_cls)
        _cache[key] = (st, per_core, nc_prog)

    ns = st['ns']
    # layer-0 table: bf16(feat * ns), core-major padded layout, replicated
    hp0_full = np.zeros((NPAD, 128), dtype=ml_dtypes.bfloat16)
    scaled = (feat * ns[:, None]).astype(ml_dtypes.bfloat16)
    for c in range(NC):
        hp0_full[c * PADSHARD:c * PADSHARD + SHARD] = \
            scaled[c * SHARD:(c + 1) * SHARD]

    iota = np.tile(np.arange(128, dtype=np.float32), (128, 1))
    W2p = np.zeros((128, fcp), dtype=np.float32)
    W2p[:, :f_cls] = np.asarray(W2, dtype=np.float32)
    b2rep = np.zeros((128, fcp), dtype=np.float32)
    b2rep[:, :f_cls] = np.asarray(b2, dtype=np.float32)[None, :]
    bfv = lambda a: np.ascontiguousarray(a).astype(ml_dtypes.bfloat16)

    in_maps = []
    for c in range(NC):
        pc = per_core[c]
        in_maps.append({
            "hp0full": hp0_full,
            "idx16": pc['idx'],
            "dstl": bfv(pc['dstl']),
            "iota": bfv(iota),
            "nsndcol": pc['nsndcol'],
            "ndcol": pc['ndcol'],
            "W0": np.asarray(W0, dtype=np.float32),
            "W1": np.asarray(W1, dtype=np.float32),
            "W2p": W2p,
            "b2rep": b2rep,
        })

    import os
    trace = os.environ.get("GCN_TRACE") == "1"
    res = run_bass_kernel_spmd(nc_prog, in_maps, core_ids=list(range(NC)),
                               trace=trace)
    global last_results
    last_results = res
    out = np.concatenate([res.results[c]["out"] for c in range(NC)], axis=0)
    return np.ascontiguousarray(out, dtype=np.float32)


last_results = None
